# revision 31
# baseline (speedup 1.0000x reference)
"""Trainium2 Bass kernel for nn_CrossAttentionNoGate (v2).

Reference computation (per MSA row s):
    q = split_heads(x_q @ wq); k = split_heads(x_kv @ wk); v = split_heads(x_kv @ wv)
    a = softmax(q k^T/sqrt(D) + (mask-1)*INF + bias)
    out = merge_heads(a @ v) @ wo + bo

Sharding: S=128 rows split 16-per-core across 8 NeuronCores (data parallel);
weights and pair bias replicated.

Per-core design (v2):
  - x^T ([2C=64*rowparity+ch, token]) is pre-transposed on the HOST and
    DMAed straight to SBUF as bf16; no xbar transposes, no hi/lo merge.
  - projections contract K=128 with row-parity zero-padded bf16 weights.
  - logits are computed transposed ([kv, q]) in bf16 (2 cols/cycle on
    the PE).  The pair bias is added PRE-exp on the PE: each QK^T PSUM
    bank's accumulation group starts with an identity-matmul of the
    bf16 bias^T, then the per-head QK^T matmuls accumulate on top.
    Heads sharing a PSUM bank {h, h+4} share a PE row-group (strict
    serial); cross-bank heads run row-tile concurrent - concurrent
    row-tiles writing the same PSUM bank crash the device.
  - softmax without max-subtraction: ONE exp per [128,1024] psum tile on
    ACT (additive mask as per-partition activation bias), writing the
    softmax weights directly as bf16.  ACT does nothing else (it is the
    theoretical bottleneck engine at ~1.26us per 1024-wide exp).
  - AV: 64-col padded v (col 63 = ones => denominator row), col-tiled at
    out bases {0,64}; kv halves back-to-back per head in one psum bank.
  - denominators gathered to 8 partitions by tiny SBUF DMAs, inverted on
    DVE, broadcast to [128,1024] f32 psum with a K=8 selector matmul;
    normalize multiply on DVE straight from psum.
  - output projection contracts the padded layout against wo_aug (zero
    rows kill pad/denominator rows): [q-block, 64] natural layout.
  - each row's tail (recip/R/normalize | out-proj) is deferred into the
    next row's qk-phase | post-AV slot, so its serial latency never
    head-of-line blocks the in-order engine queues.
"""

import math

import numpy as np

import concourse.bass as bass
import concourse.mybir as mybir
from concourse import bacc as _bacc
import concourse.tile as tile
from concourse import bass_utils

B, S, Q, KV = 1, 128, 256, 256
CQ, CKV = 64, 64
H, D = 8, 32
NCORES = 8
SC = S // NCORES
S2 = SC // 2
INF = 1.0e9
SCALE = 1.0 / math.sqrt(D)

F32 = mybir.dt.float32
F32R = mybir.dt.float32r
BF16 = mybir.dt.bfloat16
EXP = mybir.ActivationFunctionType.Exp

# moving-operand dtype for x (projection inputs): F32R (exact) or BF16 (2x)
X_DT = BF16


def _build(has_bo):
    nc = _bacc.Bacc()

    xqT = nc.declare_dram_parameter("xqT", [S2, 128, Q], X_DT, isOutput=False)
    xkT = nc.declare_dram_parameter("xkT", [S2, 128, KV], X_DT, isOutput=False)
    biasT = nc.declare_dram_parameter("biasT", [128, 2, 2, 1024], BF16, isOutput=False)
    maskcol = nc.declare_dram_parameter("maskcol", [128, SC, 2], F32, isOutput=False)
    ident = nc.declare_dram_parameter("ident", [128, 128], BF16, isOutput=False)
    esel = nc.declare_dram_parameter("esel", [8, 4, 128], BF16, isOutput=False)
    wq = nc.declare_dram_parameter("wq", [128, 2, 2, 128], BF16, isOutput=False)
    wk = nc.declare_dram_parameter("wk", [128, 2, 2, 128], BF16, isOutput=False)
    wv = nc.declare_dram_parameter("wv", [128, 2, 256], BF16, isOutput=False)
    wo3 = nc.declare_dram_parameter("wo3", [4, 128, CQ], BF16, isOutput=False)
    if has_bo:
        bo1 = nc.declare_dram_parameter("bo1", [1, CQ], F32R, isOutput=False)
    out = nc.declare_dram_parameter("out", [SC, 128, 2, CQ], F32R, isOutput=True)

    from contextlib import ExitStack

    with tile.TileContext(nc) as tc, ExitStack() as ctx:
        def pool(name, bufs, space="SBUF"):
            return ctx.enter_context(tc.tile_pool(name=name, bufs=bufs, space=space))

        singles = pool("singles", 1)
        xin = pool("xin", 2 * S2)
        qkp = pool("qk", 2)
        expabp = pool("expab", 6)
        avsbp = pool("avsb", 3)
        otnp = pool("otn", 2)
        drp = pool("dr", 4)
        finp = pool("fin", 3)
        ring = pool("ring", 3, "PSUM")
        avp = pool("avp", 1, "PSUM")

        # ---- constants (sync HWDGE ring, in dependency order: the ring
        # is FIFO and s2=0's projections need the weights first; the 1MB
        # bias tile goes last - it is only needed once QK^T tiles finish)
        wq_sb = singles.tile([128, 2, 2, 128], BF16, tag="wq")
        wk_sb = singles.tile([128, 2, 2, 128], BF16, tag="wk")
        wv_sb = singles.tile([128, 2, 256], BF16, tag="wv")
        wo_sb = singles.tile([128, 4, CQ], BF16, tag="wo")
        id_sb = singles.tile([128, 128], BF16, tag="id")
        esel_sb = singles.tile([8, 4, 128], BF16, tag="esel")
        mk_sb = singles.tile([128, SC, 2], F32, tag="mk")
        bias_sb = singles.tile([128, 2, 2, 1024], BF16, tag="biasT")
        nc.sync.dma_start(out=wq_sb[:], in_=wq[:])
        nc.sync.dma_start(out=wk_sb[:], in_=wk[:])
        nc.sync.dma_start(out=wv_sb[:], in_=wv[:])
        for t4 in range(4):
            nc.sync.dma_start(out=wo_sb[:, t4, :], in_=wo3[t4])
        nc.sync.dma_start(out=id_sb[:], in_=ident[:])
        nc.sync.dma_start(out=esel_sb[:], in_=esel[:])
        nc.sync.dma_start(out=mk_sb[:], in_=maskcol[:])
        if has_bo:
            bo_sb = singles.tile([1, CQ], F32R, tag="bo")
            ones_sb = singles.tile([1, 128], F32R, tag="ones")
            nc.sync.dma_start(out=bo_sb[:], in_=bo1[:])
            nc.vector.memset(ones_sb[:], 1.0)

        # ---- input prefetch on the second (ACT) HWDGE ring, in parallel
        # with the constants on the sync ring
        x_tiles = []
        for s2 in range(S2):
            xq_t = xin.tile([128, Q], X_DT, tag="xq")
            xk_t = xin.tile([128, KV], X_DT, tag="xk")
            nc.scalar.dma_start(out=xq_t[:], in_=xqT[s2])
            nc.scalar.dma_start(out=xk_t[:], in_=xkT[s2])
            x_tiles.append((xq_t, xk_t))
        nc.sync.dma_start(out=bias_sb[:], in_=biasT[:])

        # warm the ACT table after the input-DMA dispatch (walrus puts the
        # ~2.7us exp table load before the first ACTIVATE; here it hides
        # under the prefetch drain without delaying the DMA dispatch)
        warm_in = singles.tile([1, 8], F32, tag="warmi")
        warm_out = singles.tile([1, 8], F32, tag="warmo")
        nc.vector.memset(warm_in[:], 0.0)
        nc.scalar.activation(out=warm_out[:], in_=warm_in[:], func=EXP)

        # v tiles: one per row parity, ones column set once, d-cols
        # overwritten each s2 (cols 32..62 hold stale junk that wo_aug's
        # zero rows annihilate)
        v_sb = []
        for vi in range(4):
            vt = singles.tile([128, 2, H, 2 * D], BF16, tag=f"v{vi}")
            nc.vector.memset(vt[:, :, :, D : 2 * D - 1], 0.0)
            nc.vector.memset(vt[:, :, :, 2 * D - 1 : 2 * D], 1.0)
            v_sb.append(vt)

        # Tail of row s, deferred into row s+1 (head-of-line blocking):
        # tailA (recip/R/normalize) issues before row s+1's AV so the
        # GPSIMD normalize latency hides under it; tailB (out-proj) after.
        def make_tails(s, av_sb, d_sb):
            def tailA():
                d_f = drp.tile([H, Q], F32, tag="df")
                r_f = drp.tile([H, Q], F32, tag="r")
                r_sr = drp.tile([H, Q], BF16, tag="rr")
                nc.vector.tensor_copy(out=d_f[:], in_=d_sb[:])
                nc.vector.reciprocal_approx_fast(out=r_f[:], in_=d_f[:])
                nc.vector.tensor_copy(out=r_sr[:], in_=r_f[:])
                R_ps = ring.tile([128, 1024], F32, tag="ps")
                for t4 in range(4):
                    nc.tensor.matmul(
                        R_ps[:, Q * t4 : Q * (t4 + 1)], esel_sb[:, t4, :], r_sr[:]
                    )
                otn = otnp.tile([128, 1024], BF16, tag="otn")
                nc.vector.tensor_mul(otn[:], av_sb[:], R_ps[:])
                return otn

            def tailB(otn):
                fin_ps = ring.tile([128, 2, CQ], F32, tag="ps")
                for qc in range(2):
                    for t4 in range(4):
                        nc.tensor.matmul(
                            fin_ps[:, qc, :],
                            otn[:, Q * t4 + 128 * qc : Q * t4 + 128 * qc + 128],
                            wo_sb[:, t4, :],
                            start=(t4 == 0),
                            stop=(t4 == 3 and not has_bo),
                        )
                    if has_bo:
                        nc.tensor.matmul(
                            fin_ps[:, qc, :], ones_sb[:], bo_sb[:],
                            start=False, stop=True,
                        )
                fin_sb = finp.tile([128, 2, CQ], F32R, tag="fin")
                nc.vector.tensor_copy(out=fin_sb[:], in_=fin_ps[:])
                nc.sync.dma_start(out=out[s], in_=fin_sb[:])

            return tailA, tailB

        pending = None

        def do_proj(s2):
            # projections: K=128 (row-parity zero-padded weights)
            xq_t, xk_t = x_tiles[s2]
            # v_ps allocated first: the next qk tile reuses its ring slot,
            # and the v copies drain early (before the qT/kT copies)
            v_ps = ring.tile([128, 2, 2, 256], F32, tag="ps")
            qT_ps = ring.tile([128, 2, 2, Q], F32, tag="ps")
            kT_ps = ring.tile([128, 2, 2, KV], F32, tag="ps")
            for rp in range(2):
                for b in range(2):
                    nc.tensor.matmul(qT_ps[:, rp, b, :], wq_sb[:, rp, b, :], xq_t[:])
                    nc.tensor.matmul(kT_ps[:, rp, b, :], wk_sb[:, rp, b, :], xk_t[:])
            for rp in range(2):
                for ck in range(2):
                    nc.tensor.matmul(
                        v_ps[:, rp, ck, :],
                        xk_t[:, 128 * ck : 128 * ck + 128],
                        wv_sb[:, rp, :],
                    )
            # psum -> sbuf: rp0 halves first so row 0's QK^T never waits
            # on row 1's copies
            qT_sb = qkp.tile([128, 2, 2, Q], BF16, tag="qT")
            kT_sb = qkp.tile([128, 2, 2, 2, 128], BF16, tag="kT")
            for rp in range(2):
                for ck in range(2):
                    nc.vector.tensor_copy(
                        out=v_sb[2 * (s2 % 2) + rp][:, ck, :, 0:D],
                        in_=v_ps[:, rp, ck, :].rearrange("p (h d) -> p h d", h=H),
                    )
            for rp in range(2):
                nc.vector.tensor_copy(out=qT_sb[:, rp, :, :], in_=qT_ps[:, rp, :, :])
                nc.vector.tensor_copy(
                    out=kT_sb[:, rp, :, :, :],
                    in_=kT_ps[:, rp, :, :].rearrange("p b (ck r) -> p b ck r", ck=2),
                )
            return qT_sb, kT_sb

        # ---- main loop over row pairs
        for s2 in range(S2):
            qT_sb, kT_sb = do_proj(s2)

            for rp in range(2):
                s = 2 * s2 + rp
                expabs = {}
                # head h -> tile g2=(h%4)//2, bank bk=h%2? no: bk=(h%4)%2,
                # member m=h//4, col 512*bk+256*m, PE row-group 32*(h%4).
                # Same-bank heads {h, h+4} share a row-group (strict serial);
                # cross-bank heads run row-tile concurrent.
                for g2 in range(2):
                    for ck in range(2):
                        qk = ring.tile([128, 1024], F32, tag="ps")
                        # g2=1: bias enters the PSUM accumulation via the
                        # PE (identity matmul, bank group start); g2=0:
                        # pure logits, bias applied multiplicatively post
                        # exp on the otherwise-idle GPSIMD (exp(l+m)*exp(b))
                        pe_bias = True
                        if pe_bias:
                            for bk in range(2):
                                nc.tensor.matmul(
                                    qk[:, 512 * bk : 512 * bk + 512],
                                    id_sb[:],
                                    bias_sb[:, ck, g2, 512 * bk : 512 * bk + 512],
                                    start=True,
                                    stop=False,
                                )
                        for m in range(2):
                            for bk in range(2):
                                q4 = 2 * g2 + bk
                                nc.tensor.matmul(
                                    qk[:, 512 * bk + 256 * m : 512 * bk + 256 * m + 256],
                                    kT_sb[32 * q4 : 32 * q4 + 32, rp, m, ck, :],
                                    qT_sb[32 * q4 : 32 * q4 + 32, rp, m, :],
                                    start=(not pe_bias and m == 0),
                                    stop=(m == 1),
                                    tile_position=(32 * q4, 0),
                                )
                        expab = expabp.tile([128, 1024], BF16, tag="expab")
                        if pe_bias:
                            nc.scalar.activation(
                                out=expab[:], in_=qk[:], func=EXP,
                                bias=mk_sb[:, s, ck : ck + 1],
                            )
                        else:
                            expa = expabp.tile([128, 1024], BF16, tag="expa")
                            nc.scalar.activation(
                                out=expa[:], in_=qk[:], func=EXP,
                                bias=mk_sb[:, s, ck : ck + 1],
                            )
                            nc.gpsimd.tensor_mul(
                                expab[:], expa[:], expB_sb[:, ck, g2, :]
                            )
                        expabs[(ck, g2)] = expab

                if pending is not None:
                    ptailA, ptailB = pending
                    potn = ptailA()

                # AV: kv halves back-to-back per head; out col-tiled {0,64}.
                # The previous row's out-projection (tailB) slots between
                # the two AV groups, filling the wait for the row's last exp.
                av_ps = avp.tile([128, 1024], F32, tag="av")
                for g2 in range(2):
                    for m in range(2):
                        for bk in range(2):
                            h = 4 * m + 2 * g2 + bk
                            t4, u = h // 2, h % 2
                            for ck in range(2):
                                nc.tensor.matmul(
                                    av_ps[64 * u : 64 * u + 64, Q * t4 : Q * (t4 + 1)],
                                    v_sb[2 * (s2 % 2) + rp][:, ck, h, :],
                                    expabs[(ck, g2)][
                                        :, 512 * bk + 256 * m : 512 * bk + 256 * m + 256
                                    ],
                                    start=(ck == 0),
                                    stop=(ck == 1),
                                )

                av_sb = avsbp.tile([128, 1024], BF16, tag="avsb")
                nc.vector.tensor_copy(out=av_sb[:], in_=av_ps[:])

                # denominators (rows 63 / 127) -> 8 partitions
                d_sb = drp.tile([H, Q], BF16, tag="d")
                for u in range(2):
                    nc.sync.dma_start(
                        out=d_sb[4 * u : 4 * u + 4, :],
                        in_=av_sb[64 * u + 63 : 64 * u + 64, :],
                    )

                if pending is not None:
                    ptailB(potn)
                pending = make_tails(s, av_sb, d_sb)

        if pending is not None:
            ptailA, ptailB = pending
            ptailB(ptailA())

    nc.finalize()
    return nc


_CACHE = {}


def _get_nc(has_bo):
    if has_bo not in _CACHE:
        _CACHE[has_bo] = _build(has_bo)
    return _CACHE[has_bo]


def _host_prep(input_q, input_kv, mask, bias, wq, wk, wv, wo, bo):
    """Per-core input maps (host-side layout only)."""
    import ml_dtypes

    x_np = np.float32 if X_DT == F32R else ml_dtypes.bfloat16

    # projection weights, row-parity zero-padded to K=128
    def zpad4(w, scale=1.0):  # [64, 256] -> [128, 2, 2, 128]
        wz = np.zeros((128, 2, 2, 128), np.float32)
        for rp in range(2):
            for b in range(2):
                wz[64 * rp : 64 * rp + 64, rp, b, :] = (
                    w[:, 128 * b : 128 * b + 128] * scale
                )
        return wz

    wq_s = zpad4(wq.astype(np.float32), SCALE)
    wk_s = zpad4(wk.astype(np.float32))
    wv_s = np.zeros((128, 2, 256), np.float32)
    for rp in range(2):
        wv_s[64 * rp : 64 * rp + 64, rp, :] = wv.astype(np.float32)

    # bias^T bf16: biasT[p, ck, g2, 512*bk + 256*m + q] = bias[h=4m+2g2+bk, q, kv]
    bt = bias[0, 0].astype(np.float32)  # [H, Q, KV]
    btT = np.ascontiguousarray(bt.transpose(2, 0, 1))  # [KV, H, Q]
    btT = btT.reshape(2, 128, H, 256)  # [ck, p, h, q]
    perm = np.array([[[0, 4], [1, 5]], [[2, 6], [3, 7]]])  # [g2, bk, m] -> h
    biasT = btT[:, :, perm, :]  # [ck, p, g2, bk, m, q]
    biasT = np.ascontiguousarray(biasT.transpose(1, 0, 2, 3, 4, 5))
    biasT = biasT.reshape(128, 2, 2, 1024)

    # additive mask columns: mk[p, s_local, ck] for kv = 128*ck + p
    mterm = (mask[0, :, 0, 0, :].astype(np.float32) - 1.0) * INF  # [S, KV]
    mterm = mterm.reshape(S, 2, 128).transpose(2, 0, 1)  # [p, s, ck]

    # wo with padded-aug zero rows: wo_aug[t, 64u+j, c] = wo[(2t+u)*32+j, c], j<32
    wo_aug = np.zeros((4, 128, CQ), np.float32)
    for h in range(H):
        t4, u = h // 2, h % 2
        wo_aug[t4, 64 * u : 64 * u + D] = wo[h * D : (h + 1) * D]

    # selector: esel[r, t, 64u+j] = 1 iff r == 4u + t
    esel_h = np.zeros((8, 4, 128), np.float32)
    for t4 in range(4):
        esel_h[t4, t4, 0:64] = 1.0
        esel_h[4 + t4, t4, 64:128] = 1.0

    ident_h = np.eye(128, dtype=ml_dtypes.bfloat16)

    has_bo = bool(np.any(bo != 0))
    in_maps = []
    for i in range(NCORES):
        sl = slice(SC * i, SC * (i + 1))
        # x^T: [s2, 64*rp + ch, token]
        xq = input_q[0, sl].astype(np.float32)  # [16, Q, 64]
        xk = input_kv[0, sl].astype(np.float32)
        xqT_h = np.ascontiguousarray(
            xq.reshape(S2, 2, Q, 64).transpose(0, 1, 3, 2).reshape(S2, 128, Q)
        ).astype(x_np)
        xkT_h = np.ascontiguousarray(
            xk.reshape(S2, 2, KV, 64).transpose(0, 1, 3, 2).reshape(S2, 128, KV)
        ).astype(x_np)
        m = {
            "xqT": xqT_h,
            "xkT": xkT_h,
            "biasT": biasT.astype(ml_dtypes.bfloat16),
            "maskcol": np.ascontiguousarray(mterm[:, sl, :]),
            "ident": ident_h,
            "esel": esel_h.astype(ml_dtypes.bfloat16),
            "wq": wq_s.astype(ml_dtypes.bfloat16),
            "wk": wk_s.astype(ml_dtypes.bfloat16),
            "wv": wv_s.astype(ml_dtypes.bfloat16),
            "wo3": wo_aug.astype(ml_dtypes.bfloat16),
        }
        if has_bo:
            m["bo1"] = np.ascontiguousarray(bo.astype(np.float32).reshape(1, CQ))
        in_maps.append(m)
    return has_bo, in_maps


def kernel(input_q, input_kv, mask, bias, wq, wk, wv, wo, bo, **_):
    has_bo, in_maps = _host_prep(input_q, input_kv, mask, bias, wq, wk, wv, wo, bo)
    nc = _get_nc(has_bo)
    res = bass_utils.run_bass_kernel_spmd(nc, in_maps, core_ids=list(range(NCORES)))
    outs = []
    for i in range(NCORES):
        o = res.results[i]["out"]  # [SC, 128, 2, CQ]: (s, p, qc, c), q = 128*qc + p
        o = np.asarray(o, np.float32).transpose(0, 2, 1, 3).reshape(SC, Q, CQ)
        outs.append(o)
    full = np.concatenate(outs, axis=0).reshape(B, S, Q, CQ)
    return full.astype(np.float32)
